# revision 49
# baseline (speedup 1.0000x reference)
"""Trainium2 Bass kernel for nn_NeuralNet_19516331393457 (dense_mlp).

Pipeline: x = embed[data] (48-entry table); h1 = relu(x@W1+b1);
h2 = tanh(h1@W2+b2); out = h2@W3+b3; return out[argmax(F(out0, out1))].

Strategy (data-parallel over N=500000 on 8 cores), fp16 on device:
  - Host: tiny-table gather embed[data] in fp16 fused with a tile-blocked
    transpose; 16 up-front segment DMAs with 8 KiB DRAM lines stream the
    whole shard into SBUF (~124 KiB/partition) at full HBM bandwidth.
  - Device (per core, 63488 padded samples = 62 pairs of 2x512 chunks):
    MM1+relu and pair-packed MM2+tanh for all pairs; the tiny out=h2@W3
    is HYBRID: pairs 0..45 ship h2 (fp16 slabs, host does h2@W3 -- DRAM
    write bandwidth ~254 GB/s absorbs 6 MB while the PE works), pairs
    46..61 run MM3 on-device (their h2 is born too late to drain, so
    they go out as 0.4 MB of packed fp32->fp16 outs instead of 2 MB).
      * software-pipelined, 1-2 iteration skew; in-order PE queue sees
        only aged dependencies; stationaries grouped (w3,w2A,w2B,w1)
      * MM2 pair-packed: chunk A -> PSUM rows 0:64, chunk B -> rows
        64:128 (dup W2 at PE col groups 0/64); one tanh (+b2 stacked)
        evicts both chunks
      * device MM3 pair-packed twice: stacked h2 x block-diag W3 slab ->
        [4,512] at PSUM partition base {0,32,64}; one bank collects 3
        pairs; 6 copies total
      * relu/copies statically balanced between DVE and ACT
  - Host: decode both out paths, out = h2 @ W3 in fp32 for the slab part,
    F in float64, global argmax, return out[argmax] + b3.
"""

import numpy as np

import concourse.mybir as mybir
import concourse.tile as tile
from concourse import bacc
from concourse.bass_utils import run_bass_kernel_spmd

N = 500000
D = 128
H1 = 128
H2 = 64
NCLS = 2
NCORES = 8
CHUNK = 512
NPC_RAW = N // NCORES              # 62500 samples per core
PAIRS = 62                         # pairs of 2 chunks per core
CHUNKS = 2 * PAIRS                 # 124 chunks
NPC = CHUNKS * CHUNK               # 63488 padded samples per core
SEGPAIRS = 8                       # pairs per input DMA segment
SEGS = -(-PAIRS // SEGPAIRS)       # 8 segments (16 KiB DRAM lines each)
DEVPAIRS = 0                       # on-device MM3 disabled (see doc)
HOSTPAIRS = PAIRS - DEVPAIRS       # all h2 ships to the host
SLABPAIRS = 8                      # pairs per output slab
SLABS = -(-HOSTPAIRS // SLABPAIRS)  # 8 slabs (last holds 6 pairs)
HOSTTANH = 52                      # pairs >= this ship pre-tanh h2 (DVE
                                   # bias-add evict; host applies tanh)

_F16 = mybir.dt.float16
_F32 = mybir.dt.float32

# measured per-op eviction costs (ns) for static DVE/ACT load balancing
_DVE_RELU, _ACT_RELU = 700.0, 590.0
_DVE_COPY, _ACT_COPY = 700.0, 650.0
_ACT_TANH = 638.0


def _build_bass():
    nc = bacc.Bacc(
        "TRN2",
        target_bir_lowering=False,
        debug=False,
        enable_asserts=False,
        num_devices=NCORES,
    )
    x_t = nc.dram_tensor("x_t", [SEGS, D, SEGPAIRS * 2 * CHUNK], _F16,
                         kind="ExternalInput")
    w1 = nc.dram_tensor("w1", [D, H1], _F16, kind="ExternalInput")
    w2d = nc.dram_tensor("w2d", [H1, 2 * H2], _F16, kind="ExternalInput")
    b1 = nc.dram_tensor("b1", [H1, 1], _F32, kind="ExternalInput")
    b2s = nc.dram_tensor("b2s", [2 * H2, 1], _F32, kind="ExternalInput")
    out_d = nc.dram_tensor("out_d", [SLABS, 128, SLABPAIRS * CHUNK], _F16,
                           kind="ExternalOutput")

    load = {"dve": 0.0, "act": 0.0}

    with tile.TileContext(nc) as tc:
        with (
            tc.tile_pool(name="w", bufs=1) as wpool,
            tc.tile_pool(name="x", bufs=SEGS) as xpool,
            tc.tile_pool(name="h1", bufs=8) as h1pool,
            tc.tile_pool(name="st", bufs=SLABS) as stpool,
            tc.tile_pool(name="p1", bufs=4, space="PSUM") as p1pool,
            tc.tile_pool(name="p2", bufs=4, space="PSUM") as p2pool,
        ):
            w1sb = wpool.tile([D, H1], _F16)
            nc.sync.dma_start(w1sb[:], w1[:, :])
            w2sb = wpool.tile([H1, 2 * H2], _F16)
            nc.sync.dma_start(w2sb[:], w2d[:, :])
            b1sb = wpool.tile([H1, 1], _F32)
            nc.sync.dma_start(b1sb[:], b1[:, :])
            b2sb = wpool.tile([2 * H2, 1], _F32)
            nc.sync.dma_start(b2sb[:], b2s[:, :])

            # prefetch the ACT table set (relu/tanh share one) under the
            # first input DMAs
            warm = wpool.tile([H1, 1], _F32)
            nc.scalar.activation(warm[:], b1sb[:],
                                 mybir.ActivationFunctionType.Relu)

            xsegs = [
                xpool.tile([D, SEGPAIRS * 2 * CHUNK], _F16,
                           name=f"xseg{s}", tag="xt")
                for s in range(SEGS)
            ]

            def issue_seg(s):
                xt = xsegs[s]
                if s == 0:
                    # per-pair slices so the first MM1 starts as soon as
                    # pair 0 lands, not after the whole 2 MiB segment
                    for q in range(SEGPAIRS):
                        sl = slice(q * 2 * CHUNK, (q + 1) * 2 * CHUNK)
                        nc.sync.dma_start(xt[:, sl], x_t[s, :, sl])
                else:
                    # last segment holds only 6 real pairs: skip the pad
                    w = min(SEGPAIRS, PAIRS - s * SEGPAIRS) * 2 * CHUNK
                    nc.sync.dma_start(xt[:, 0:w], x_t[s, :, 0:w])

            for s in range(SEGS):
                issue_seg(s)

            slabs = [
                stpool.tile([128, SLABPAIRS * CHUNK], _F16,
                            name=f"slab{o}", tag="st")
                for o in range(SLABS)
            ]

            p1s = {}     # chunk -> [128, 512] f32 psum
            h1s = {}     # chunk -> [128, 512] f16
            p2s = {}     # pair -> [128, 512] f32 psum (A rows 0:64, B 64:128)

            def emit_mm1(p):
                seg, off = divmod(p, SEGPAIRS)
                xt = xsegs[seg]
                for half in range(2):
                    c = 2 * p + half
                    p1 = p1pool.tile([H1, CHUNK], _F32, name=f"p1_{c}",
                                     tag="p1")
                    base = off * 2 * CHUNK + half * CHUNK
                    nc.tensor.matmul(
                        p1[:], w1sb[:], xt[:, base:base + CHUNK],
                        start=True, stop=True)
                    p1s[c] = p1

            def emit_relu(p):
                for half in range(2):
                    c = 2 * p + half
                    h1t = h1pool.tile([H1, CHUNK], _F16, name=f"h1_{c}",
                                      tag="h1")
                    if load["act"] + _ACT_RELU <= load["dve"] + _DVE_RELU:
                        load["act"] += _ACT_RELU
                        nc.scalar.activation(
                            h1t[:], p1s[c][:],
                            mybir.ActivationFunctionType.Relu, bias=b1sb[:])
                    else:
                        load["dve"] += _DVE_RELU
                        nc.vector.tensor_scalar(
                            h1t[:], p1s[c][:], b1sb[:], 0.0,
                            mybir.AluOpType.add, mybir.AluOpType.max)
                    h1s[c] = h1t
                    del p1s[c]

            def emit_mm2(p, col):
                # col 0: chunk A -> rows 0:64; col 1: chunk B -> rows 64:128
                if col == 0:
                    p2 = p2pool.tile([128, CHUNK], _F32, name=f"p2_{p}",
                                     tag="p2")
                    p2s[p] = p2
                nc.tensor.matmul(
                    p2s[p][col * H2:(col + 1) * H2, :],
                    w2sb[:, col * H2:(col + 1) * H2],
                    h1s[2 * p + col][:], start=True, stop=True)

            def emit_tanh(p):
                o, j = divmod(p, SLABPAIRS)
                dst = slabs[o][:, j * CHUNK:(j + 1) * CHUNK]
                if p >= HOSTTANH:
                    # evict p2+b2 on DVE; host applies tanh.  Keeps the
                    # ACT queue clear of the ops that gate final slabs.
                    load["dve"] += _DVE_RELU
                    nc.vector.tensor_scalar(
                        dst, p2s[p][:], b2sb[:], 0.0,
                        mybir.AluOpType.add, mybir.AluOpType.bypass)
                else:
                    load["act"] += _ACT_TANH
                    nc.scalar.activation(
                        dst, p2s[p][:], mybir.ActivationFunctionType.Tanh,
                        bias=b2sb[:])
                del p2s[p]
                if o == SLABS - 1:
                    # final slab: per-pair DMAs so the kernel tail is one
                    # 128 KiB transfer, not the whole slab
                    sl = slice(j * CHUNK, (j + 1) * CHUNK)
                    nc.sync.dma_start(out_d[o, :, sl], slabs[o][:, sl])
                elif o == SLABS - 2:
                    # penultimate slab in halves: its first 4 pairs drain
                    # while the last pairs are still being produced
                    if j == SLABPAIRS // 2 - 1 or j == SLABPAIRS - 1:
                        sl = slice((j - 3) * CHUNK, (j + 1) * CHUNK)
                        nc.sync.dma_start(out_d[o, :, sl], slabs[o][:, sl])
                elif j == SLABPAIRS - 1:
                    nc.sync.dma_start(out_d[o, :, :], slabs[o][:])

            # batches of 2 pairs; MM2s grouped across 2 batches (8
            # matmuls per 2 stationary loads), MM1 batches of 4 per load;
            # 1-2 iteration skew keeps the in-order PE queue off fresh
            # deps.  Late in the run relus prefer DVE so ACT's tanh tail
            # (which gates the last output slabs) finishes sooner.
            NB = PAIRS // 2
            mm2_done = set()
            for b in range(NB + 3):
                grp = []
                # group MM2s across 2 batches, except the final batch
                # runs solo one iteration earlier (shorter tail chain)
                if b >= 2 and (b % 2 == 0 or b - 1 == NB - 1):
                    grp = [bb for bb in (b - 2, b - 1)
                           if 0 <= bb < NB and bb not in mm2_done]
                    mm2_done.update(grp)
                    for col in range(2):
                        for bb in grp:
                            emit_mm2(2 * bb, col)
                            emit_mm2(2 * bb + 1, col)
                if b < NB:
                    emit_mm1(2 * b)
                    emit_mm1(2 * b + 1)
                for bb in grp:
                    emit_tanh(2 * bb)
                    emit_tanh(2 * bb + 1)
                    del h1s[4 * bb], h1s[4 * bb + 1]
                    del h1s[4 * bb + 2], h1s[4 * bb + 3]
                if b < NB:
                    emit_relu(2 * b)
                    emit_relu(2 * b + 1)

    nc.compile()
    return nc


_NC_CACHE = None


def _get_nc():
    global _NC_CACHE
    if _NC_CACHE is None:
        _NC_CACHE = _build_bass()
    return _NC_CACHE


def _weight_tensors(W1, b1, W2, b2, W3=None):
    w1 = np.ascontiguousarray(W1, dtype=np.float16)
    w2dm = np.concatenate([W2, W2], axis=1).astype(np.float16)
    b1c = np.ascontiguousarray(b1, dtype=np.float32).reshape(H1, 1)
    b2sc = np.concatenate([b2, b2]).astype(np.float32).reshape(2 * H2, 1)
    return {"w1": w1, "w2d": np.ascontiguousarray(w2dm),
            "b1": b1c, "b2s": b2sc}


def _core_inmap(data, table16, core, weights):
    npad = SEGS * SEGPAIRS * 2 * CHUNK
    dshard = data[core * NPC_RAW:(core + 1) * NPC_RAW]
    dpad = np.zeros((npad, D), dtype=dshard.dtype)
    dpad[:NPC_RAW] = dshard
    # fused fp16 gather + tile-blocked transpose: [SEGS, D, SEGPAIRS*1024]
    xt = np.ascontiguousarray(
        table16[dpad.reshape(SEGS, SEGPAIRS * 2 * CHUNK, D)
                .transpose(0, 2, 1)]
    )
    return {"x_t": xt, **weights}


def _decode_core(arr, W3f):
    """[SLABS, 128, SLABPAIRS*CHUNK] f16 h2 slabs -> [NPC, 2] f32 outs."""
    h2 = (arr.reshape(SLABS, 2, H2, SLABPAIRS, CHUNK)
          .transpose(0, 3, 1, 2, 4)
          .reshape(SLABS * SLABPAIRS, 2, H2, CHUNK)[:PAIRS]
          .astype(np.float32))
    h2[HOSTTANH:] = np.tanh(h2[HOSTTANH:])   # device shipped pre-tanh
    out = np.einsum("phfs,fc->phsc", h2, W3f, optimize=True)
    return out.reshape(NPC, NCLS)


def _F64(x, y):
    return (
        3.0 * (1.0 - x) ** 2 * np.exp(-(x**2) - (y + 1.0) ** 2)
        - 10.0 * (x / 5.0 - x**3 - y**5) * np.exp(-(x**2) - y**2)
        - 1.0 / (3.0 ** np.exp(-((x + 1.0) ** 2) - y**2))
    )


def kernel(data, embed, W1, b1, W2, b2, W3, b3):
    data = np.asarray(data)
    table16 = np.asarray(embed, dtype=np.float32).reshape(-1).astype(
        np.float16)
    W3f = np.asarray(W3, dtype=np.float32)
    b3c = np.asarray(b3, dtype=np.float32).reshape(NCLS)

    nc = _get_nc()
    weights = _weight_tensors(W1, b1, W2, b2, W3)
    in_maps = [_core_inmap(data, table16, c, weights) for c in range(NCORES)]

    res = run_bass_kernel_spmd(nc, in_maps, core_ids=list(range(NCORES)))

    outs = []
    for c in range(NCORES):
        outs.append(
            _decode_core(res.results[c]["out_d"], W3f)[:NPC_RAW])
    out_all = np.concatenate(outs, axis=0) + b3c  # [N, 2] fp32

    x64 = out_all[:, 0].astype(np.float64)
    y64 = out_all[:, 1].astype(np.float64)
    pred = _F64(x64, y64)
    idx = int(np.argmax(pred))
    return out_all[idx].astype(np.float32)


# revision 50
# speedup vs baseline: 1.0477x; 1.0477x over previous
"""Trainium2 Bass kernel for nn_NeuralNet_19516331393457 (dense_mlp).

Pipeline: x = embed[data] (48-entry table); h1 = relu(x@W1+b1);
h2 = tanh(h1@W2+b2); out = h2@W3+b3; return out[argmax(F(out0, out1))].

Strategy (data-parallel over N=500000 on 8 cores), fp16 on device:
  - Host: tiny-table gather embed[data] in fp16 fused with a tile-blocked
    transpose; 16 up-front segment DMAs with 8 KiB DRAM lines stream the
    whole shard into SBUF (~124 KiB/partition) at full HBM bandwidth.
  - Device (per core, 63488 padded samples = 62 pairs of 2x512 chunks):
    MM1+relu and pair-packed MM2+tanh for all pairs; the tiny out=h2@W3
    is HYBRID: pairs 0..45 ship h2 (fp16 slabs, host does h2@W3 -- DRAM
    write bandwidth ~254 GB/s absorbs 6 MB while the PE works), pairs
    46..61 run MM3 on-device (their h2 is born too late to drain, so
    they go out as 0.4 MB of packed fp32->fp16 outs instead of 2 MB).
      * software-pipelined, 1-2 iteration skew; in-order PE queue sees
        only aged dependencies; stationaries grouped (w3,w2A,w2B,w1)
      * MM2 pair-packed: chunk A -> PSUM rows 0:64, chunk B -> rows
        64:128 (dup W2 at PE col groups 0/64); one tanh (+b2 stacked)
        evicts both chunks
      * device MM3 pair-packed twice: stacked h2 x block-diag W3 slab ->
        [4,512] at PSUM partition base {0,32,64}; one bank collects 3
        pairs; 6 copies total
      * relu/copies statically balanced between DVE and ACT
  - Host: decode both out paths, out = h2 @ W3 in fp32 for the slab part,
    F in float64, global argmax, return out[argmax] + b3.
"""

import numpy as np

import concourse.mybir as mybir
import concourse.tile as tile
from concourse import bacc
from concourse.bass_utils import run_bass_kernel_spmd

N = 500000
D = 128
H1 = 128
H2 = 64
NCLS = 2
NCORES = 8
CHUNK = 512
NPC_RAW = N // NCORES              # 62500 samples per core
PAIRS = 62                         # pairs of 2 chunks per core
CHUNKS = 2 * PAIRS                 # 124 chunks
NPC = CHUNKS * CHUNK               # 63488 padded samples per core
SEGPAIRS = 8                       # pairs per input DMA segment
SEGS = -(-PAIRS // SEGPAIRS)       # 8 segments (16 KiB DRAM lines each)
DEVPAIRS = 0                       # on-device MM3 disabled (see doc)
HOSTPAIRS = PAIRS - DEVPAIRS       # all h2 ships to the host
SLABPAIRS = 8                      # pairs per output slab
SLABS = -(-HOSTPAIRS // SLABPAIRS)  # 8 slabs (last holds 6 pairs)
HOSTTANH = 48                      # pairs >= this ship pre-tanh h2 (DVE
                                   # bias-add evict; host applies tanh)

_F16 = mybir.dt.float16
_F32 = mybir.dt.float32

# measured per-op eviction costs (ns) for static DVE/ACT load balancing
_DVE_RELU, _ACT_RELU = 700.0, 590.0
_DVE_COPY, _ACT_COPY = 700.0, 650.0
_ACT_TANH = 638.0


def _build_bass():
    nc = bacc.Bacc(
        "TRN2",
        target_bir_lowering=False,
        debug=False,
        enable_asserts=False,
        num_devices=NCORES,
    )
    x_t = nc.dram_tensor("x_t", [SEGS, D, SEGPAIRS * 2 * CHUNK], _F16,
                         kind="ExternalInput")
    w1 = nc.dram_tensor("w1", [D, H1], _F16, kind="ExternalInput")
    w2d = nc.dram_tensor("w2d", [H1, 2 * H2], _F16, kind="ExternalInput")
    b1 = nc.dram_tensor("b1", [H1, 1], _F32, kind="ExternalInput")
    b2s = nc.dram_tensor("b2s", [2 * H2, 1], _F32, kind="ExternalInput")
    out_d = nc.dram_tensor("out_d", [SLABS, 128, SLABPAIRS * CHUNK], _F16,
                           kind="ExternalOutput")

    load = {"dve": 0.0, "act": 0.0}

    with tile.TileContext(nc) as tc:
        with (
            tc.tile_pool(name="w", bufs=1) as wpool,
            tc.tile_pool(name="x", bufs=SEGS) as xpool,
            tc.tile_pool(name="h1", bufs=8) as h1pool,
            tc.tile_pool(name="st", bufs=SLABS) as stpool,
            tc.tile_pool(name="p1", bufs=4, space="PSUM") as p1pool,
            tc.tile_pool(name="p2", bufs=4, space="PSUM") as p2pool,
        ):
            w1sb = wpool.tile([D, H1], _F16)
            nc.sync.dma_start(w1sb[:], w1[:, :])
            w2sb = wpool.tile([H1, 2 * H2], _F16)
            nc.sync.dma_start(w2sb[:], w2d[:, :])
            b1sb = wpool.tile([H1, 1], _F32)
            nc.sync.dma_start(b1sb[:], b1[:, :])
            b2sb = wpool.tile([2 * H2, 1], _F32)
            nc.sync.dma_start(b2sb[:], b2s[:, :])

            # prefetch the ACT table set (relu/tanh share one) under the
            # first input DMAs
            warm = wpool.tile([H1, 1], _F32)
            nc.scalar.activation(warm[:], b1sb[:],
                                 mybir.ActivationFunctionType.Relu)

            xsegs = [
                xpool.tile([D, SEGPAIRS * 2 * CHUNK], _F16,
                           name=f"xseg{s}", tag="xt")
                for s in range(SEGS)
            ]

            def issue_seg(s):
                xt = xsegs[s]
                if s == 0:
                    # per-pair slices so the first MM1 starts as soon as
                    # pair 0 lands, not after the whole 2 MiB segment
                    for q in range(SEGPAIRS):
                        sl = slice(q * 2 * CHUNK, (q + 1) * 2 * CHUNK)
                        nc.sync.dma_start(xt[:, sl], x_t[s, :, sl])
                else:
                    # last segment holds only 6 real pairs: skip the pad
                    w = min(SEGPAIRS, PAIRS - s * SEGPAIRS) * 2 * CHUNK
                    nc.sync.dma_start(xt[:, 0:w], x_t[s, :, 0:w])

            for s in range(SEGS):
                issue_seg(s)

            slabs = [
                stpool.tile([128, SLABPAIRS * CHUNK], _F16,
                            name=f"slab{o}", tag="st")
                for o in range(SLABS)
            ]

            p1s = {}     # chunk -> [128, 512] f32 psum
            h1s = {}     # chunk -> [128, 512] f16
            p2s = {}     # pair -> [128, 512] f32 psum (A rows 0:64, B 64:128)

            def emit_mm1(p):
                seg, off = divmod(p, SEGPAIRS)
                xt = xsegs[seg]
                for half in range(2):
                    c = 2 * p + half
                    p1 = p1pool.tile([H1, CHUNK], _F32, name=f"p1_{c}",
                                     tag="p1")
                    base = off * 2 * CHUNK + half * CHUNK
                    nc.tensor.matmul(
                        p1[:], w1sb[:], xt[:, base:base + CHUNK],
                        start=True, stop=True)
                    p1s[c] = p1

            def emit_relu(p):
                for half in range(2):
                    c = 2 * p + half
                    h1t = h1pool.tile([H1, CHUNK], _F16, name=f"h1_{c}",
                                      tag="h1")
                    if load["act"] + _ACT_RELU <= load["dve"] + _DVE_RELU:
                        load["act"] += _ACT_RELU
                        nc.scalar.activation(
                            h1t[:], p1s[c][:],
                            mybir.ActivationFunctionType.Relu, bias=b1sb[:])
                    else:
                        load["dve"] += _DVE_RELU
                        nc.vector.tensor_scalar(
                            h1t[:], p1s[c][:], b1sb[:], 0.0,
                            mybir.AluOpType.add, mybir.AluOpType.max)
                    h1s[c] = h1t
                    del p1s[c]

            def emit_mm2(p, col):
                # col 0: chunk A -> rows 0:64; col 1: chunk B -> rows 64:128
                if col == 0:
                    p2 = p2pool.tile([128, CHUNK], _F32, name=f"p2_{p}",
                                     tag="p2")
                    p2s[p] = p2
                nc.tensor.matmul(
                    p2s[p][col * H2:(col + 1) * H2, :],
                    w2sb[:, col * H2:(col + 1) * H2],
                    h1s[2 * p + col][:], start=True, stop=True)

            def emit_tanh(p):
                o, j = divmod(p, SLABPAIRS)
                dst = slabs[o][:, j * CHUNK:(j + 1) * CHUNK]
                if p >= HOSTTANH:
                    # evict p2+b2 on DVE; host applies tanh.  Keeps the
                    # ACT queue clear of the ops that gate final slabs.
                    load["dve"] += _DVE_RELU
                    nc.vector.tensor_scalar(
                        dst, p2s[p][:], b2sb[:], 0.0,
                        mybir.AluOpType.add, mybir.AluOpType.bypass)
                else:
                    load["act"] += _ACT_TANH
                    nc.scalar.activation(
                        dst, p2s[p][:], mybir.ActivationFunctionType.Tanh,
                        bias=b2sb[:])
                del p2s[p]
                if o == SLABS - 1:
                    # final slab: per-pair DMAs so the kernel tail is one
                    # 128 KiB transfer, not the whole slab
                    sl = slice(j * CHUNK, (j + 1) * CHUNK)
                    nc.sync.dma_start(out_d[o, :, sl], slabs[o][:, sl])
                elif o == SLABS - 2:
                    # penultimate slab in halves: its first 4 pairs drain
                    # while the last pairs are still being produced
                    if j == SLABPAIRS // 2 - 1 or j == SLABPAIRS - 1:
                        sl = slice((j - 3) * CHUNK, (j + 1) * CHUNK)
                        nc.sync.dma_start(out_d[o, :, sl], slabs[o][:, sl])
                elif j == SLABPAIRS - 1:
                    nc.sync.dma_start(out_d[o, :, :], slabs[o][:])

            # batches of 2 pairs; MM2s grouped across 2 batches (8
            # matmuls per 2 stationary loads), MM1 batches of 4 per load;
            # 1-2 iteration skew keeps the in-order PE queue off fresh
            # deps.  Late in the run relus prefer DVE so ACT's tanh tail
            # (which gates the last output slabs) finishes sooner.
            NB = PAIRS // 2
            mm2_done = set()
            for b in range(NB + 3):
                grp = []
                # group MM2s across 2 batches, except the final batch
                # runs solo one iteration earlier (shorter tail chain)
                if b >= 2 and (b % 2 == 0 or b - 1 == NB - 1):
                    grp = [bb for bb in (b - 2, b - 1)
                           if 0 <= bb < NB and bb not in mm2_done]
                    mm2_done.update(grp)
                    for col in range(2):
                        for bb in grp:
                            emit_mm2(2 * bb, col)
                            emit_mm2(2 * bb + 1, col)
                if b < NB:
                    emit_mm1(2 * b)
                    emit_mm1(2 * b + 1)
                for bb in grp:
                    emit_tanh(2 * bb)
                    emit_tanh(2 * bb + 1)
                    del h1s[4 * bb], h1s[4 * bb + 1]
                    del h1s[4 * bb + 2], h1s[4 * bb + 3]
                if b < NB:
                    emit_relu(2 * b)
                    emit_relu(2 * b + 1)

    nc.compile()
    return nc


_NC_CACHE = None


def _get_nc():
    global _NC_CACHE
    if _NC_CACHE is None:
        _NC_CACHE = _build_bass()
    return _NC_CACHE


def _weight_tensors(W1, b1, W2, b2, W3=None):
    w1 = np.ascontiguousarray(W1, dtype=np.float16)
    w2dm = np.concatenate([W2, W2], axis=1).astype(np.float16)
    b1c = np.ascontiguousarray(b1, dtype=np.float32).reshape(H1, 1)
    b2sc = np.concatenate([b2, b2]).astype(np.float32).reshape(2 * H2, 1)
    return {"w1": w1, "w2d": np.ascontiguousarray(w2dm),
            "b1": b1c, "b2s": b2sc}


def _core_inmap(data, table16, core, weights):
    npad = SEGS * SEGPAIRS * 2 * CHUNK
    dshard = data[core * NPC_RAW:(core + 1) * NPC_RAW]
    dpad = np.zeros((npad, D), dtype=dshard.dtype)
    dpad[:NPC_RAW] = dshard
    # fused fp16 gather + tile-blocked transpose: [SEGS, D, SEGPAIRS*1024]
    xt = np.ascontiguousarray(
        table16[dpad.reshape(SEGS, SEGPAIRS * 2 * CHUNK, D)
                .transpose(0, 2, 1)]
    )
    return {"x_t": xt, **weights}


def _decode_core(arr, W3f):
    """[SLABS, 128, SLABPAIRS*CHUNK] f16 h2 slabs -> [NPC, 2] f32 outs."""
    h2 = (arr.reshape(SLABS, 2, H2, SLABPAIRS, CHUNK)
          .transpose(0, 3, 1, 2, 4)
          .reshape(SLABS * SLABPAIRS, 2, H2, CHUNK)[:PAIRS]
          .astype(np.float32))
    h2[HOSTTANH:] = np.tanh(h2[HOSTTANH:])   # device shipped pre-tanh
    out = np.einsum("phfs,fc->phsc", h2, W3f, optimize=True)
    return out.reshape(NPC, NCLS)


def _F64(x, y):
    return (
        3.0 * (1.0 - x) ** 2 * np.exp(-(x**2) - (y + 1.0) ** 2)
        - 10.0 * (x / 5.0 - x**3 - y**5) * np.exp(-(x**2) - y**2)
        - 1.0 / (3.0 ** np.exp(-((x + 1.0) ** 2) - y**2))
    )


def kernel(data, embed, W1, b1, W2, b2, W3, b3):
    data = np.asarray(data)
    table16 = np.asarray(embed, dtype=np.float32).reshape(-1).astype(
        np.float16)
    W3f = np.asarray(W3, dtype=np.float32)
    b3c = np.asarray(b3, dtype=np.float32).reshape(NCLS)

    nc = _get_nc()
    weights = _weight_tensors(W1, b1, W2, b2, W3)
    in_maps = [_core_inmap(data, table16, c, weights) for c in range(NCORES)]

    res = run_bass_kernel_spmd(nc, in_maps, core_ids=list(range(NCORES)))

    outs = []
    for c in range(NCORES):
        outs.append(
            _decode_core(res.results[c]["out_d"], W3f)[:NPC_RAW])
    out_all = np.concatenate(outs, axis=0) + b3c  # [N, 2] fp32

    x64 = out_all[:, 0].astype(np.float64)
    y64 = out_all[:, 1].astype(np.float64)
    pred = _F64(x64, y64)
    idx = int(np.argmax(pred))
    return out_all[idx].astype(np.float32)


# revision 51
# speedup vs baseline: 1.0485x; 1.0008x over previous
"""Trainium2 Bass kernel for nn_NeuralNet_19516331393457 (dense_mlp).

Pipeline: x = embed[data] (48-entry table); h1 = relu(x@W1+b1);
h2 = tanh(h1@W2+b2); out = h2@W3+b3; return out[argmax(F(out0, out1))].

Strategy (data-parallel over N=500000 on 8 cores), fp16 on device:
  - Host: tiny-table gather embed[data] in fp16 fused with a tile-blocked
    transpose; 16 up-front segment DMAs with 8 KiB DRAM lines stream the
    whole shard into SBUF (~124 KiB/partition) at full HBM bandwidth.
  - Device (per core, 63488 padded samples = 62 pairs of 2x512 chunks):
    MM1+relu and pair-packed MM2+tanh for all pairs; the tiny out=h2@W3
    is HYBRID: pairs 0..45 ship h2 (fp16 slabs, host does h2@W3 -- DRAM
    write bandwidth ~254 GB/s absorbs 6 MB while the PE works), pairs
    46..61 run MM3 on-device (their h2 is born too late to drain, so
    they go out as 0.4 MB of packed fp32->fp16 outs instead of 2 MB).
      * software-pipelined, 1-2 iteration skew; in-order PE queue sees
        only aged dependencies; stationaries grouped (w3,w2A,w2B,w1)
      * MM2 pair-packed: chunk A -> PSUM rows 0:64, chunk B -> rows
        64:128 (dup W2 at PE col groups 0/64); one tanh (+b2 stacked)
        evicts both chunks
      * device MM3 pair-packed twice: stacked h2 x block-diag W3 slab ->
        [4,512] at PSUM partition base {0,32,64}; one bank collects 3
        pairs; 6 copies total
      * relu/copies statically balanced between DVE and ACT
  - Host: decode both out paths, out = h2 @ W3 in fp32 for the slab part,
    F in float64, global argmax, return out[argmax] + b3.
"""

import numpy as np

import concourse.mybir as mybir
import concourse.tile as tile
from concourse import bacc
from concourse.bass_utils import run_bass_kernel_spmd

N = 500000
D = 128
H1 = 128
H2 = 64
NCLS = 2
NCORES = 8
CHUNK = 512
NPC_RAW = N // NCORES              # 62500 samples per core
PAIRS = 62                         # pairs of 2 chunks per core
CHUNKS = 2 * PAIRS                 # 124 chunks
NPC = CHUNKS * CHUNK               # 63488 padded samples per core
SEGPAIRS = 8                       # pairs per input DMA segment
SEGS = -(-PAIRS // SEGPAIRS)       # 8 segments (16 KiB DRAM lines each)
DEVPAIRS = 0                       # on-device MM3 disabled (see doc)
HOSTPAIRS = PAIRS - DEVPAIRS       # all h2 ships to the host
SLABPAIRS = 8                      # pairs per output slab
SLABS = -(-HOSTPAIRS // SLABPAIRS)  # 8 slabs (last holds 6 pairs)
HOSTTANH = 48                      # pairs >= this ship pre-tanh h2 (DVE
                                   # bias-add evict; host applies tanh)

_F16 = mybir.dt.float16
_F32 = mybir.dt.float32

# measured per-op eviction costs (ns) for static DVE/ACT load balancing
_DVE_RELU, _ACT_RELU = 700.0, 590.0
_DVE_COPY, _ACT_COPY = 700.0, 650.0
_ACT_TANH = 638.0


def _build_bass():
    nc = bacc.Bacc(
        "TRN2",
        target_bir_lowering=False,
        debug=False,
        enable_asserts=False,
        num_devices=NCORES,
    )
    x_t = nc.dram_tensor("x_t", [SEGS, D, SEGPAIRS * 2 * CHUNK], _F16,
                         kind="ExternalInput")
    w1 = nc.dram_tensor("w1", [D, H1], _F16, kind="ExternalInput")
    w2d = nc.dram_tensor("w2d", [H1, 2 * H2], _F16, kind="ExternalInput")
    b1 = nc.dram_tensor("b1", [H1, 1], _F32, kind="ExternalInput")
    b2s = nc.dram_tensor("b2s", [2 * H2, 1], _F32, kind="ExternalInput")
    out_d = nc.dram_tensor("out_d", [SLABS, 128, SLABPAIRS * CHUNK], _F16,
                           kind="ExternalOutput")

    load = {"dve": 0.0, "act": 0.0}

    with tile.TileContext(nc) as tc:
        with (
            tc.tile_pool(name="w", bufs=1) as wpool,
            tc.tile_pool(name="x", bufs=SEGS) as xpool,
            tc.tile_pool(name="h1", bufs=8) as h1pool,
            tc.tile_pool(name="st", bufs=SLABS) as stpool,
            tc.tile_pool(name="p1", bufs=4, space="PSUM") as p1pool,
            tc.tile_pool(name="p2", bufs=4, space="PSUM") as p2pool,
        ):
            w1sb = wpool.tile([D, H1], _F16)
            nc.sync.dma_start(w1sb[:], w1[:, :])
            w2sb = wpool.tile([H1, 2 * H2], _F16)
            nc.sync.dma_start(w2sb[:], w2d[:, :])
            b1sb = wpool.tile([H1, 1], _F32)
            nc.sync.dma_start(b1sb[:], b1[:, :])
            b2sb = wpool.tile([2 * H2, 1], _F32)
            nc.sync.dma_start(b2sb[:], b2s[:, :])

            # prefetch the ACT table set (relu/tanh share one) under the
            # first input DMAs
            warm = wpool.tile([H1, 1], _F32)
            nc.scalar.activation(warm[:], b1sb[:],
                                 mybir.ActivationFunctionType.Relu)

            xsegs = [
                xpool.tile([D, SEGPAIRS * 2 * CHUNK], _F16,
                           name=f"xseg{s}", tag="xt")
                for s in range(SEGS)
            ]

            def issue_seg(s):
                xt = xsegs[s]
                if s == 0:
                    # per-pair slices so the first MM1 starts as soon as
                    # pair 0 lands, not after the whole 2 MiB segment
                    for q in range(SEGPAIRS):
                        sl = slice(q * 2 * CHUNK, (q + 1) * 2 * CHUNK)
                        nc.sync.dma_start(xt[:, sl], x_t[s, :, sl])
                else:
                    # last segment holds only 6 real pairs: skip the pad
                    w = min(SEGPAIRS, PAIRS - s * SEGPAIRS) * 2 * CHUNK
                    nc.sync.dma_start(xt[:, 0:w], x_t[s, :, 0:w])

            for s in range(SEGS):
                issue_seg(s)

            slabs = [
                stpool.tile([128, SLABPAIRS * CHUNK], _F16,
                            name=f"slab{o}", tag="st")
                for o in range(SLABS)
            ]

            p1s = {}     # chunk -> [128, 512] f32 psum
            h1s = {}     # chunk -> [128, 512] f16
            p2s = {}     # pair -> [128, 512] f32 psum (A rows 0:64, B 64:128)

            def emit_mm1(p):
                seg, off = divmod(p, SEGPAIRS)
                xt = xsegs[seg]
                for half in range(2):
                    c = 2 * p + half
                    p1 = p1pool.tile([H1, CHUNK], _F32, name=f"p1_{c}",
                                     tag="p1")
                    base = off * 2 * CHUNK + half * CHUNK
                    nc.tensor.matmul(
                        p1[:], w1sb[:], xt[:, base:base + CHUNK],
                        start=True, stop=True)
                    p1s[c] = p1

            def emit_relu(p):
                for half in range(2):
                    c = 2 * p + half
                    h1t = h1pool.tile([H1, CHUNK], _F16, name=f"h1_{c}",
                                      tag="h1")
                    if load["act"] + _ACT_RELU <= load["dve"] + _DVE_RELU:
                        load["act"] += _ACT_RELU
                        nc.scalar.activation(
                            h1t[:], p1s[c][:],
                            mybir.ActivationFunctionType.Relu, bias=b1sb[:])
                    else:
                        load["dve"] += _DVE_RELU
                        nc.vector.tensor_scalar(
                            h1t[:], p1s[c][:], b1sb[:], 0.0,
                            mybir.AluOpType.add, mybir.AluOpType.max)
                    h1s[c] = h1t
                    del p1s[c]

            def emit_mm2(p, col):
                # col 0: chunk A -> rows 0:64; col 1: chunk B -> rows 64:128
                if col == 0:
                    p2 = p2pool.tile([128, CHUNK], _F32, name=f"p2_{p}",
                                     tag="p2")
                    p2s[p] = p2
                nc.tensor.matmul(
                    p2s[p][col * H2:(col + 1) * H2, :],
                    w2sb[:, col * H2:(col + 1) * H2],
                    h1s[2 * p + col][:], start=True, stop=True)

            def emit_tanh(p):
                o, j = divmod(p, SLABPAIRS)
                dst = slabs[o][:, j * CHUNK:(j + 1) * CHUNK]
                if p >= HOSTTANH:
                    # evict p2+b2 on DVE; host applies tanh.  Keeps the
                    # ACT queue clear of the ops that gate final slabs.
                    load["dve"] += _DVE_RELU
                    nc.vector.tensor_scalar(
                        dst, p2s[p][:], b2sb[:], 0.0,
                        mybir.AluOpType.add, mybir.AluOpType.bypass)
                else:
                    load["act"] += _ACT_TANH
                    nc.scalar.activation(
                        dst, p2s[p][:], mybir.ActivationFunctionType.Tanh,
                        bias=b2sb[:])
                del p2s[p]
                if o >= SLABS - 3:
                    # last three slabs: per-pair DMAs.  A whole-slab
                    # trigger waits for the slab's LAST eviction and
                    # head-of-line blocks the sync queue, stalling later
                    # slabs whose data is already resident.
                    sl = slice(j * CHUNK, (j + 1) * CHUNK)
                    nc.sync.dma_start(out_d[o, :, sl], slabs[o][:, sl])
                elif j == SLABPAIRS - 1:
                    nc.sync.dma_start(out_d[o, :, :], slabs[o][:])

            # batches of 2 pairs; MM2s grouped across 2 batches (8
            # matmuls per 2 stationary loads), MM1 batches of 4 per load;
            # 1-2 iteration skew keeps the in-order PE queue off fresh
            # deps.  Late in the run relus prefer DVE so ACT's tanh tail
            # (which gates the last output slabs) finishes sooner.
            NB = PAIRS // 2
            mm2_done = set()
            for b in range(NB + 3):
                grp = []
                # group MM2s across 2 batches, except the final batch
                # runs solo one iteration earlier (shorter tail chain)
                if b >= 2 and (b % 2 == 0 or b - 1 == NB - 1):
                    grp = [bb for bb in (b - 2, b - 1)
                           if 0 <= bb < NB and bb not in mm2_done]
                    mm2_done.update(grp)
                    for col in range(2):
                        for bb in grp:
                            emit_mm2(2 * bb, col)
                            emit_mm2(2 * bb + 1, col)
                if b < NB:
                    emit_mm1(2 * b)
                    emit_mm1(2 * b + 1)
                for bb in grp:
                    emit_tanh(2 * bb)
                    emit_tanh(2 * bb + 1)
                    del h1s[4 * bb], h1s[4 * bb + 1]
                    del h1s[4 * bb + 2], h1s[4 * bb + 3]
                if b < NB:
                    emit_relu(2 * b)
                    emit_relu(2 * b + 1)

    nc.compile()
    return nc


_NC_CACHE = None


def _get_nc():
    global _NC_CACHE
    if _NC_CACHE is None:
        _NC_CACHE = _build_bass()
    return _NC_CACHE


def _weight_tensors(W1, b1, W2, b2, W3=None):
    w1 = np.ascontiguousarray(W1, dtype=np.float16)
    w2dm = np.concatenate([W2, W2], axis=1).astype(np.float16)
    b1c = np.ascontiguousarray(b1, dtype=np.float32).reshape(H1, 1)
    b2sc = np.concatenate([b2, b2]).astype(np.float32).reshape(2 * H2, 1)
    return {"w1": w1, "w2d": np.ascontiguousarray(w2dm),
            "b1": b1c, "b2s": b2sc}


def _core_inmap(data, table16, core, weights):
    npad = SEGS * SEGPAIRS * 2 * CHUNK
    dshard = data[core * NPC_RAW:(core + 1) * NPC_RAW]
    dpad = np.zeros((npad, D), dtype=dshard.dtype)
    dpad[:NPC_RAW] = dshard
    # fused fp16 gather + tile-blocked transpose: [SEGS, D, SEGPAIRS*1024]
    xt = np.ascontiguousarray(
        table16[dpad.reshape(SEGS, SEGPAIRS * 2 * CHUNK, D)
                .transpose(0, 2, 1)]
    )
    return {"x_t": xt, **weights}


def _decode_core(arr, W3f):
    """[SLABS, 128, SLABPAIRS*CHUNK] f16 h2 slabs -> [NPC, 2] f32 outs."""
    h2 = (arr.reshape(SLABS, 2, H2, SLABPAIRS, CHUNK)
          .transpose(0, 3, 1, 2, 4)
          .reshape(SLABS * SLABPAIRS, 2, H2, CHUNK)[:PAIRS]
          .astype(np.float32))
    h2[HOSTTANH:] = np.tanh(h2[HOSTTANH:])   # device shipped pre-tanh
    out = np.einsum("phfs,fc->phsc", h2, W3f, optimize=True)
    return out.reshape(NPC, NCLS)


def _F64(x, y):
    return (
        3.0 * (1.0 - x) ** 2 * np.exp(-(x**2) - (y + 1.0) ** 2)
        - 10.0 * (x / 5.0 - x**3 - y**5) * np.exp(-(x**2) - y**2)
        - 1.0 / (3.0 ** np.exp(-((x + 1.0) ** 2) - y**2))
    )


def kernel(data, embed, W1, b1, W2, b2, W3, b3):
    data = np.asarray(data)
    table16 = np.asarray(embed, dtype=np.float32).reshape(-1).astype(
        np.float16)
    W3f = np.asarray(W3, dtype=np.float32)
    b3c = np.asarray(b3, dtype=np.float32).reshape(NCLS)

    nc = _get_nc()
    weights = _weight_tensors(W1, b1, W2, b2, W3)
    in_maps = [_core_inmap(data, table16, c, weights) for c in range(NCORES)]

    res = run_bass_kernel_spmd(nc, in_maps, core_ids=list(range(NCORES)))

    outs = []
    for c in range(NCORES):
        outs.append(
            _decode_core(res.results[c]["out_d"], W3f)[:NPC_RAW])
    out_all = np.concatenate(outs, axis=0) + b3c  # [N, 2] fp32

    x64 = out_all[:, 0].astype(np.float64)
    y64 = out_all[:, 1].astype(np.float64)
    pred = _F64(x64, y64)
    idx = int(np.argmax(pred))
    return out_all[idx].astype(np.float32)
